# revision 18
# baseline (speedup 1.0000x reference)
"""Trainium2 Bass kernel: 3x3 VALID conv2d, stride 1.

Full input [32, 64, 112, 112] f32 + weights [128, 64, 3, 3] f32
-> output [32, 128, 110, 110] f32.

Data-parallel across 8 NeuronCores: 4 images per core.

Per-core formulation: conv as PE matmuls, out = lhsT.T @ rhs with
K (contraction, partitions) = 128 = two (ky,kx) taps x 64 channels,
M (out partitions) = 128 output channels,
N (moving free dim) = 4 input-width rows = 448 (<= 512, one PSUM bank).
The 2 rightmost columns of each 112-wide row are conv garbage; the
PSUM->SBUF copy compacts to the valid 110 columns.

Tap fusion (5 matmuls per chunk instead of the naive 9):
  tile1 layout: partitions 0..63 = image rows h (shift (0,0)),
          partitions 64..127 = rows h+1 (shift (1,0)).
          m=0..2 fuse taps (0,kx)+(1,kx), rhs offset y0*W+kx.
  tile2 layout: partitions 0..63 = shift (2,0), 64..127 = shift (2,1).
          m=3 fuses taps (2,0)+(2,1) at offset y0*W; m=4 is the lone
          (2,2) tap from half A at offset y0*W+2 (half-B weights zero).
Both layouts are prepared host-side (host prep is free; only HW time
counts) and streamed as full-128-partition DMAs.

Inputs are fp16 (full-rate PE streaming, fp32 PSUM accumulation);
output is written fp16 and upcast on the host; rel err ~5e-4.

DMA model (measured): all dma_starts drain through the same 16 SDMA
engines. SWDGE (gpsimd) writes descriptor rings into SBUF parts 0-31
(contends with compute) and its engine boots ~5us late, so everything
rides the two HWDGE rings instead: t1 + weights on qSPDynamicHW
(nc.sync), t2 + output stores on qActDynamicHW (nc.scalar), each ring
FIFO in issue order. Issue order therefore IS the delivery schedule:
inputs stream as halo'd BAND tiles (~30 rows, 2-row halo so every
chunk's rhs window lives in one band) issued in consumption order;
images 2,3 reuse the 8-buf band pools so their loads self-pace via
per-band WAR. Each HWDGE dma_start costs ~0.65us of sequencer issue
time -- keep DMA count low. SBUF->DRAM runs ~4x slower per byte than
DRAM->SBUF regardless of descriptor size or dtype, so the fp16 output
(vs fp32) is what keeps the store stream off the critical path.

Compute schedule: chunk-major in interleaved pairs (m0c0 m0c1 .. m4c1)
so consecutive matmuls alternate PSUM banks and each chunk's PSUM
drains right after its stop matmul (weight reloads are per-matmul and
hide under the 190ns stream). All PSUM->SBUF copies on vector (scalar
and sync sequencers must stay free to pump their DMA rings; gpsimd has
no PSUM path). Output staging: 56-row blocks for images 0-2, 8-row
blocks for image 3 so the final drain is shallow.
"""

import numpy as np

B_FULL = 32
N_CORES = 8
B_CORE = B_FULL // N_CORES  # 4 images per core
C_IN = 64
C_OUT = 128
H = W = 112
OH = OW = 110
XCOLS = 12322  # max col accessed: 108*112 + 2 + 224
BAND_ROWS = 28  # band pitch in rows; tiles carry a 2-row halo
N_BANDS = 4
T1B = 30 * W  # t1 band tile cols (30 rows; last band partial)
T2B = 3140  # x2 band tile cols (max in-band read is 3138)

_NC = None


def _build():
    from contextlib import ExitStack

    import concourse.tile as tile
    from concourse import bacc, mybir

    nc = bacc.Bacc("TRN2", target_bir_lowering=False, debug=False)
    x = nc.dram_tensor(
        "x", [B_CORE, 128, XCOLS], mybir.dt.float16, kind="ExternalInput"
    )
    x2 = nc.dram_tensor(
        "x2", [B_CORE, 128, XCOLS], mybir.dt.float16, kind="ExternalInput"
    )
    w = nc.dram_tensor("w", [128, 5, 128], mybir.dt.float16, kind="ExternalInput")
    y = nc.dram_tensor(
        "y", [B_CORE, C_OUT, OH * OW], mybir.dt.float16, kind="ExternalOutput"
    )

    chunks = []
    for b in range(B_CORE):
        y0 = 0
        for r in [4] * 27 + [2]:  # 27*4 + 2 = 110 output rows
            chunks.append((b, y0, r))
            y0 += r
    assert len(chunks) % 4 == 0

    # output staging blocks: rows [0,56) and [56,110) of each image
    OBLK = [(0, 56), (56, 110)]

    with tile.TileContext(nc) as tc, ExitStack() as ctx:
        t1pool = ctx.enter_context(tc.tile_pool(name="t1p", bufs=8))
        t2pool = ctx.enter_context(tc.tile_pool(name="t2p", bufs=8))
        wpool = ctx.enter_context(tc.tile_pool(name="wp", bufs=1))
        opool = ctx.enter_context(tc.tile_pool(name="op", bufs=4))
        ppool = ctx.enter_context(tc.tile_pool(name="pp", bufs=8, space="PSUM"))

        wt = wpool.tile([128, 5, 128], mybir.dt.float16)
        nc.scalar.dma_start(wt[:], w.ap())

        xa = x.ap()
        x2a = x2.ap()
        ya = y.ap()

        t1bands = {}  # (b, k) -> (tile, col offset)
        t2bands = {}

        def issue_band_loads(b, k):
            lo1 = k * BAND_ROWS * W
            hi1 = min(lo1 + T1B, XCOLS)
            t1 = t1pool.tile([128, T1B], mybir.dt.float16, tag="t1")
            t2 = t2pool.tile([128, T2B], mybir.dt.float16, tag="t2")
            lo2 = k * BAND_ROWS * W
            hi2 = min(lo2 + T2B, XCOLS)
            # t1 rides the sync ring, t2 + weights the scalar ring: two
            # rings double early delivery bandwidth and halve issue
            # serialization (each HWDGE dma_start costs ~0.65us of
            # sequencer time)
            if b == 0 and k == 0:
                # fine leading slices so the PE starts sooner
                nc.sync.dma_start(t1[:, 0 : 8 * W], xa[b][:, 0 : 8 * W])
                nc.scalar.dma_start(t2[:, 0 : 8 * W], x2a[b][:, 0 : 8 * W])
                nc.sync.dma_start(
                    t1[:, 8 * W : hi1 - lo1], xa[b][:, 8 * W : hi1]
                )
                nc.scalar.dma_start(
                    t2[:, 8 * W : hi2 - lo2], x2a[b][:, 8 * W : hi2]
                )
            else:
                nc.sync.dma_start(t1[:, 0 : hi1 - lo1], xa[b][:, lo1:hi1])
                nc.scalar.dma_start(t2[:, 0 : hi2 - lo2], x2a[b][:, lo2:hi2])
            t1bands[(b, k)] = (t1, lo1)
            t2bands[(b, k)] = (t2, lo2)

        def issue_chunk_pair(c):
            """Two chunks, matmuls interleaved to alternate PSUM banks."""
            pair = chunks[c : c + 2]
            pts = [
                ppool.tile([128, 448], mybir.dt.float32, name="pt", tag="pt")
                for _ in pair
            ]
            for m in range(5):
                for (b, y0, rows), pt in zip(pair, pts):
                    n = rows * W
                    k = y0 // BAND_ROWS
                    if m < 3:
                        t, lo = t1bands[(b, k)]
                        j = y0 * W + m - lo
                    elif m == 3:
                        t, lo = t2bands[(b, k)]
                        j = y0 * W - lo
                    else:
                        t, lo = t2bands[(b, k)]
                        j = y0 * W + 2 - lo
                    nc.tensor.matmul(
                        pt[:, 0:n],
                        wt[:, m, :],
                        t[:, j : j + n],
                        start=(m == 0),
                        stop=(m == 4),
                        skip_group_check=True,
                    )
            return pair, pts

        def issue_image(b, blocks):
            for r0, r1 in blocks:
                ot = opool.tile([128, 56 * OW], mybir.dt.float16, tag="ot")
                off = 0
                for c in range(b * 28 + r0 // 4, b * 28 + (r1 + 3) // 4, 2):
                    pair, pts = issue_chunk_pair(c)
                    for (bb, y0, rows), pt in zip(pair, pts):
                        psrc = pt[:].rearrange("p (r c) -> p r c", c=W)
                        odst = ot[:, off : off + rows * OW].rearrange(
                            "p (r c) -> p r c", c=OW
                        )
                        nc.vector.tensor_copy(
                            odst[:, 0:rows], psrc[:, 0:rows, 0:OW]
                        )
                        off += rows * OW
                assert off == (r1 - r0) * OW
                nc.scalar.dma_start(
                    ya[b][:, r0 * OW : r1 * OW],
                    ot[:, 0 : (r1 - r0) * OW],
                )

        # images 0,1 fill all 8 band buffers up front; images 2,3 reuse
        # them, so their loads are issued after the prior image's
        # compute (the WAR dep is per-band: image b's band k only waits
        # for image b-2's band k readers, ~2 images ahead of need).
        FINE = [(r, min(r + 8, OH)) for r in range(0, OH, 8)]
        for b in (0, 1):
            for k in range(N_BANDS):
                issue_band_loads(b, k)
        issue_image(0, OBLK)
        for k in range(N_BANDS):
            issue_band_loads(2, k)
        issue_image(1, OBLK)
        for k in range(N_BANDS):
            issue_band_loads(3, k)
        issue_image(2, OBLK)
        issue_image(3, FINE)

    nc.compile()
    return nc


def _get_nc():
    global _NC
    if _NC is None:
        _NC = _build()
    return _NC


def _prep_weights(weights: np.ndarray) -> np.ndarray:
    # m=0..2: w5[ci, m, co] = w[co, ci, 0, m];  w5[64+ci, m, co] = w[co, ci, 1, m]
    # m=3:    w5[ci, 3, co] = w[co, ci, 2, 0];  w5[64+ci, 3, co] = w[co, ci, 2, 1]
    # m=4:    w5[ci, 4, co] = w[co, ci, 2, 2];  w5[64+ci, 4, co] = 0
    w = np.asarray(weights, dtype=np.float32)
    wt = w.transpose(1, 2, 3, 0)  # [ci, ky, kx, co]
    w5 = np.zeros((128, 5, 128), np.float32)
    w5[0:64, 0:3, :] = wt[:, 0, :, :]
    w5[64:128, 0:3, :] = wt[:, 1, :, :]
    w5[0:64, 3, :] = wt[:, 2, 0, :]
    w5[64:128, 3, :] = wt[:, 2, 1, :]
    w5[0:64, 4, :] = wt[:, 2, 2, :]
    return w5.astype(np.float16)


def kernel(input_image: np.ndarray, weights: np.ndarray, _trace: bool = False):
    from concourse.bass_utils import run_bass_kernel_spmd

    nc = _get_nc()
    x16 = np.asarray(input_image).astype(np.float16)  # [32, 64, 112, 112]
    # tile1 layout: [b, s*64+ci, h*112+w], s=0 -> row h, s=1 -> row h+1
    xd = np.zeros((B_FULL, 128, XCOLS), np.float16)
    flat = x16.reshape(B_FULL, C_IN, H * W)
    xd[:, :C_IN] = flat[:, :, :XCOLS]
    xd[:, C_IN:] = flat[:, :, W : W + XCOLS]
    # tile2 layout: halves are shifts (2,0) and (2,1) of the image
    xp = np.zeros((B_FULL, C_IN, H + 2, W + 1), np.float16)
    xp[:, :, :H, :W] = x16
    x2d = np.empty((B_FULL, 128, XCOLS), np.float16)
    x2d[:, :C_IN] = xp[:, :, 2 : 2 + 111, 0:W].reshape(B_FULL, C_IN, -1)[
        :, :, :XCOLS
    ]
    x2d[:, C_IN:] = xp[:, :, 2 : 2 + 111, 1 : 1 + W].reshape(B_FULL, C_IN, -1)[
        :, :, :XCOLS
    ]
    w5 = _prep_weights(weights)
    in_maps = [
        {
            "x": xd[B_CORE * i : B_CORE * (i + 1)],
            "x2": x2d[B_CORE * i : B_CORE * (i + 1)],
            "w": w5,
        }
        for i in range(N_CORES)
    ]
    res = run_bass_kernel_spmd(
        nc, in_maps, core_ids=list(range(N_CORES)), trace=_trace
    )
    out = np.concatenate(
        [
            res.results[i]["y"]
            .reshape(B_CORE, C_OUT, OH, OW)
            .astype(np.float32)
            for i in range(N_CORES)
        ],
        axis=0,
    )
    if _trace:
        return out, res
    return out


# revision 19
# speedup vs baseline: 1.0584x; 1.0584x over previous
"""Trainium2 Bass kernel: 3x3 VALID conv2d, stride 1.

Full input [32, 64, 112, 112] f32 + weights [128, 64, 3, 3] f32
-> output [32, 128, 110, 110] f32.

Data-parallel across 8 NeuronCores: 4 images per core.

Per-core formulation: conv as PE matmuls, out = lhsT.T @ rhs with
K (contraction, partitions) = 128 = two (ky,kx) taps x 64 channels,
M (out partitions) = 128 output channels,
N (moving free dim) = 4 input-width rows = 448 (<= 512, one PSUM bank).
The 2 rightmost columns of each 112-wide row are conv garbage; the
PSUM->SBUF copy compacts to the valid 110 columns.

Tap fusion (5 matmuls per chunk instead of the naive 9):
  tile1 layout: partitions 0..63 = image rows h (shift (0,0)),
          partitions 64..127 = rows h+1 (shift (1,0)).
          m=0..2 fuse taps (0,kx)+(1,kx), rhs offset y0*W+kx.
  tile2 layout: partitions 0..63 = shift (2,0), 64..127 = shift (2,1).
          m=3 fuses taps (2,0)+(2,1) at offset y0*W; m=4 is the lone
          (2,2) tap from half A at offset y0*W+2 (half-B weights zero).
Both layouts are prepared host-side (host prep is free; only HW time
counts) and streamed as full-128-partition DMAs.

Inputs are fp16 (full-rate PE streaming, fp32 PSUM accumulation);
output is written fp16 and upcast on the host; rel err ~5e-4.

DMA model (measured): all dma_starts drain through the same 16 SDMA
engines. SWDGE (gpsimd) writes descriptor rings into SBUF parts 0-31
(contends with compute) and its engine boots ~5us late, so everything
rides the two HWDGE rings instead: t1 + weights on qSPDynamicHW
(nc.sync), t2 + output stores on qActDynamicHW (nc.scalar), each ring
FIFO in issue order. Issue order therefore IS the delivery schedule:
inputs stream as halo'd BAND tiles (~30 rows, 2-row halo so every
chunk's rhs window lives in one band) issued in consumption order;
images 2,3 reuse the 8-buf band pools so their loads self-pace via
per-band WAR. Each HWDGE dma_start costs ~0.65us of sequencer issue
time -- keep DMA count low. SBUF->DRAM runs ~4x slower per byte than
DRAM->SBUF regardless of descriptor size or dtype, so the fp16 output
(vs fp32) is what keeps the store stream off the critical path.

Compute schedule: chunk-major in interleaved pairs (m0c0 m0c1 .. m4c1)
so consecutive matmuls alternate PSUM banks and each chunk's PSUM
drains right after its stop matmul (weight reloads are per-matmul and
hide under the 190ns stream). All PSUM->SBUF copies on vector (scalar
and sync sequencers must stay free to pump their DMA rings; gpsimd has
no PSUM path). Output staging: 56-row blocks for images 0-2, 8-row
blocks for image 3 so the final drain is shallow.
"""

import numpy as np

B_FULL = 32
N_CORES = 8
B_CORE = B_FULL // N_CORES  # 4 images per core
C_IN = 64
C_OUT = 128
H = W = 112
OH = OW = 110
XCOLS = 12322  # max col accessed: 108*112 + 2 + 224
BAND_ROWS = 28  # band pitch in rows; tiles carry a 2-row halo
N_BANDS = 4
T1B = 30 * W  # t1 band tile cols (30 rows; last band partial)
T2B = 3140  # x2 band tile cols (max in-band read is 3138)

_NC = None


def _build():
    from contextlib import ExitStack

    import concourse.tile as tile
    from concourse import bacc, mybir

    nc = bacc.Bacc("TRN2", target_bir_lowering=False, debug=False)
    x = nc.dram_tensor(
        "x", [B_CORE, 128, XCOLS], mybir.dt.float16, kind="ExternalInput"
    )
    x2 = nc.dram_tensor(
        "x2", [B_CORE, 128, XCOLS], mybir.dt.float16, kind="ExternalInput"
    )
    w = nc.dram_tensor("w", [128, 5, 128], mybir.dt.float16, kind="ExternalInput")
    y = nc.dram_tensor(
        "y", [B_CORE, C_OUT, OH * OW], mybir.dt.float16, kind="ExternalOutput"
    )

    chunks = []
    for b in range(B_CORE):
        y0 = 0
        for r in [4] * 27 + [2]:  # 27*4 + 2 = 110 output rows
            chunks.append((b, y0, r))
            y0 += r
    assert len(chunks) % 4 == 0

    # output staging blocks: rows [0,56) and [56,110) of each image
    OBLK = [(0, 56), (56, 110)]

    with tile.TileContext(nc) as tc, ExitStack() as ctx:
        t1pool = ctx.enter_context(tc.tile_pool(name="t1p", bufs=8))
        t2pool = ctx.enter_context(tc.tile_pool(name="t2p", bufs=12))
        wpool = ctx.enter_context(tc.tile_pool(name="wp", bufs=1))
        opool = ctx.enter_context(tc.tile_pool(name="op", bufs=4))
        ppool = ctx.enter_context(tc.tile_pool(name="pp", bufs=8, space="PSUM"))

        wt = wpool.tile([128, 5, 128], mybir.dt.float16)
        nc.scalar.dma_start(wt[:], w.ap())

        xa = x.ap()
        x2a = x2.ap()
        ya = y.ap()

        t1bands = {}  # (b, k) -> (tile, col offset)
        t2bands = {}

        def issue_band_loads(b, k):
            lo1 = k * BAND_ROWS * W
            hi1 = min(lo1 + T1B, XCOLS)
            t1 = t1pool.tile([128, T1B], mybir.dt.float16, tag="t1")
            t2 = t2pool.tile([128, T2B], mybir.dt.float16, tag="t2")
            lo2 = k * BAND_ROWS * W
            hi2 = min(lo2 + T2B, XCOLS)
            # t1 rides the sync ring, t2 + weights the scalar ring: two
            # rings double early delivery bandwidth and halve issue
            # serialization (each HWDGE dma_start costs ~0.65us of
            # sequencer time)
            if b == 0 and k == 0:
                # fine leading slices so the PE starts sooner
                nc.sync.dma_start(t1[:, 0 : 8 * W], xa[b][:, 0 : 8 * W])
                nc.scalar.dma_start(t2[:, 0 : 8 * W], x2a[b][:, 0 : 8 * W])
                nc.sync.dma_start(
                    t1[:, 8 * W : hi1 - lo1], xa[b][:, 8 * W : hi1]
                )
                nc.scalar.dma_start(
                    t2[:, 8 * W : hi2 - lo2], x2a[b][:, 8 * W : hi2]
                )
            else:
                nc.sync.dma_start(t1[:, 0 : hi1 - lo1], xa[b][:, lo1:hi1])
                nc.scalar.dma_start(t2[:, 0 : hi2 - lo2], x2a[b][:, lo2:hi2])
            t1bands[(b, k)] = (t1, lo1)
            t2bands[(b, k)] = (t2, lo2)

        def issue_chunk_pair(c):
            """Two chunks, matmuls interleaved to alternate PSUM banks."""
            pair = chunks[c : c + 2]
            pts = [
                ppool.tile([128, 448], mybir.dt.float32, name="pt", tag="pt")
                for _ in pair
            ]
            for m in range(5):
                for (b, y0, rows), pt in zip(pair, pts):
                    n = rows * W
                    k = y0 // BAND_ROWS
                    if m < 3:
                        t, lo = t1bands[(b, k)]
                        j = y0 * W + m - lo
                    elif m == 3:
                        t, lo = t2bands[(b, k)]
                        j = y0 * W - lo
                    else:
                        t, lo = t2bands[(b, k)]
                        j = y0 * W + 2 - lo
                    nc.tensor.matmul(
                        pt[:, 0:n],
                        wt[:, m, :],
                        t[:, j : j + n],
                        start=(m == 0),
                        stop=(m == 4),
                        skip_group_check=True,
                    )
            return pair, pts

        def issue_image(b, blocks):
            for r0, r1 in blocks:
                ot = opool.tile([128, 56 * OW], mybir.dt.float16, tag="ot")
                off = 0
                for c in range(b * 28 + r0 // 4, b * 28 + (r1 + 3) // 4, 2):
                    pair, pts = issue_chunk_pair(c)
                    for (bb, y0, rows), pt in zip(pair, pts):
                        psrc = pt[:].rearrange("p (r c) -> p r c", c=W)
                        odst = ot[:, off : off + rows * OW].rearrange(
                            "p (r c) -> p r c", c=OW
                        )
                        nc.vector.tensor_copy(
                            odst[:, 0:rows], psrc[:, 0:rows, 0:OW]
                        )
                        off += rows * OW
                assert off == (r1 - r0) * OW
                nc.scalar.dma_start(
                    ya[b][:, r0 * OW : r1 * OW],
                    ot[:, 0 : (r1 - r0) * OW],
                )

        # images 0,1 fill all 8 band buffers up front; images 2,3 reuse
        # them, so their loads are issued after the prior image's
        # compute (the WAR dep is per-band: image b's band k only waits
        # for image b-2's band k readers, ~2 images ahead of need).
        FINE = [(r, min(r + 8, OH)) for r in range(0, OH, 8)]
        for b in (0, 1):
            for k in range(N_BANDS):
                issue_band_loads(b, k)
        issue_image(0, OBLK)
        # images 2,3 load issues go ahead of image 1's outputs on the
        # scalar ring: with 12 t2 buffers their WAR waits release by
        # ~36us (image-0 readers), so the last image's data is on chip
        # ~40us before its compute window instead of racing it.
        for k in range(N_BANDS):
            issue_band_loads(2, k)
        for k in range(N_BANDS):
            issue_band_loads(3, k)
        issue_image(1, OBLK)
        issue_image(2, OBLK)
        issue_image(3, FINE)

    nc.compile()
    return nc


def _get_nc():
    global _NC
    if _NC is None:
        _NC = _build()
    return _NC


def _prep_weights(weights: np.ndarray) -> np.ndarray:
    # m=0..2: w5[ci, m, co] = w[co, ci, 0, m];  w5[64+ci, m, co] = w[co, ci, 1, m]
    # m=3:    w5[ci, 3, co] = w[co, ci, 2, 0];  w5[64+ci, 3, co] = w[co, ci, 2, 1]
    # m=4:    w5[ci, 4, co] = w[co, ci, 2, 2];  w5[64+ci, 4, co] = 0
    w = np.asarray(weights, dtype=np.float32)
    wt = w.transpose(1, 2, 3, 0)  # [ci, ky, kx, co]
    w5 = np.zeros((128, 5, 128), np.float32)
    w5[0:64, 0:3, :] = wt[:, 0, :, :]
    w5[64:128, 0:3, :] = wt[:, 1, :, :]
    w5[0:64, 3, :] = wt[:, 2, 0, :]
    w5[64:128, 3, :] = wt[:, 2, 1, :]
    w5[0:64, 4, :] = wt[:, 2, 2, :]
    return w5.astype(np.float16)


def kernel(input_image: np.ndarray, weights: np.ndarray, _trace: bool = False):
    from concourse.bass_utils import run_bass_kernel_spmd

    nc = _get_nc()
    x16 = np.asarray(input_image).astype(np.float16)  # [32, 64, 112, 112]
    # tile1 layout: [b, s*64+ci, h*112+w], s=0 -> row h, s=1 -> row h+1
    xd = np.zeros((B_FULL, 128, XCOLS), np.float16)
    flat = x16.reshape(B_FULL, C_IN, H * W)
    xd[:, :C_IN] = flat[:, :, :XCOLS]
    xd[:, C_IN:] = flat[:, :, W : W + XCOLS]
    # tile2 layout: halves are shifts (2,0) and (2,1) of the image
    xp = np.zeros((B_FULL, C_IN, H + 2, W + 1), np.float16)
    xp[:, :, :H, :W] = x16
    x2d = np.empty((B_FULL, 128, XCOLS), np.float16)
    x2d[:, :C_IN] = xp[:, :, 2 : 2 + 111, 0:W].reshape(B_FULL, C_IN, -1)[
        :, :, :XCOLS
    ]
    x2d[:, C_IN:] = xp[:, :, 2 : 2 + 111, 1 : 1 + W].reshape(B_FULL, C_IN, -1)[
        :, :, :XCOLS
    ]
    w5 = _prep_weights(weights)
    in_maps = [
        {
            "x": xd[B_CORE * i : B_CORE * (i + 1)],
            "x2": x2d[B_CORE * i : B_CORE * (i + 1)],
            "w": w5,
        }
        for i in range(N_CORES)
    ]
    res = run_bass_kernel_spmd(
        nc, in_maps, core_ids=list(range(N_CORES)), trace=_trace
    )
    out = np.concatenate(
        [
            res.results[i]["y"]
            .reshape(B_CORE, C_OUT, OH, OW)
            .astype(np.float32)
            for i in range(N_CORES)
        ],
        axis=0,
    )
    if _trace:
        return out, res
    return out
